# revision 12
# baseline (speedup 1.0000x reference)
"""Trainium2 Bass kernel for a causal multi-head attention layer.

Full (unsharded) contract:
    hidden_states [2, 2048, 2048] f32, W_attn [2048, 6144], b_attn [6144],
    W_proj [2048, 2048], b_proj [2048]  ->  out [2, 2048, 2048] f32

Sharding over 8 NeuronCores: core = b * 4 + g where b in {0,1} is the batch
(data parallel) and g in 0..3 is a group of 4 heads (tensor parallel over
heads: c_attn output columns / c_proj input rows split by head). Each core
computes a full [2048, 2048] c_proj partial (transposed); the host sums the
4 head-group partials per batch and adds b_proj.

On-device dataflow is fully transposed so no transposes are ever required:
  phase A: qT,kT [c, s] = W.T @ xT (transposed projection), v [s, c] (plain
           projection)  -- both consume xT [d, s] and W [d, c] naturally.
  phase B: scoresT [t, s] = kT_blk.T @ qT ; exp ; causal 0/1 mask multiply ;
           outT_un [d, s] += v_blk.T @ expT ; denominator via ones-matmul
           (broadcast column sum) ; outT = outT_un * reciprocal.
  phase C: partialT [Dout, s] = Wp_blk.T @ outT, interleaved with B per
           s-tile so PE/ACT/DVE load stays balanced.
All matmuls are float32r (full PE rate at N>=256, ~1.5e-4 relative error).
"""

from contextlib import ExitStack

import numpy as np

B = 2
S = 2048
D = 2048
NUM_HEAD = 16
HD = 128
N_CORES = 8
HEADS_PER_CORE = 4  # head-group size (512 columns)
CG = HEADS_PER_CORE * HD  # 512

_compiled = None


def _build_program():
    import concourse.bacc as bacc
    import concourse.mybir as mybir
    import concourse.tile as tile

    F32 = mybir.dt.float32
    F32R = mybir.dt.float32r
    ACT = mybir.ActivationFunctionType
    ALU = mybir.AluOpType

    nc = bacc.Bacc("TRN2", target_bir_lowering=False, debug=False)

    xT = nc.dram_tensor("xT", [D, S], F32R, kind="ExternalInput").ap()
    wqkv = nc.dram_tensor("wqkv", [D, 3 * CG], F32R, kind="ExternalInput").ap()
    bqkv = nc.dram_tensor("bqkv", [2 * CG], F32, kind="ExternalInput").ap()
    vbias = nc.dram_tensor("vbias", [128, CG], F32, kind="ExternalInput").ap()
    wproj = nc.dram_tensor("wproj", [CG, D], F32R, kind="ExternalInput").ap()
    masks = nc.dram_tensor("masks", [4 * 128, 512], F32R, kind="ExternalInput").ap()
    ones = nc.dram_tensor("ones", [128, 128], F32R, kind="ExternalInput").ap()
    outT = nc.dram_tensor("outT", [D, S], F32, kind="ExternalOutput").ap()

    DB = D // 128      # 16 contraction blocks
    A_ST = 512         # phase A s-tile width
    A_NT = S // A_ST   # 4
    QKC = 2 * CG // 128  # 8 q/k column blocks of 128

    with tile.TileContext(nc) as tc:
        with ExitStack() as octx:
            # pools that live across phases
            vpool = octx.enter_context(tc.tile_pool(name="vsb", bufs=1))
            consts = octx.enter_context(tc.tile_pool(name="consts", bufs=1))
            dram = octx.enter_context(tc.tile_pool(name="dram", bufs=1, space="DRAM"))

            v_sb = vpool.tile([128, DB * CG], F32R)  # [s%128, sblk-major x c] 4MB
            # per-column-block DRAM staging for qT/kT rows (fine-grained deps
            # so phase B loads can start as soon as each row block completes)
            qk_d = []
            for cb in range(QKC):
                qk_cb = dram.tile([128, S], F32R, tag=f"qk{cb}", name=f"qk_d{cb}")
                qk_d.append(qk_cb)

            ones_sb = consts.tile([128, 128], F32R)
            nc.sync.dma_start(ones_sb[:], ones[:])
            vb_sb = consts.tile([128, CG], F32)
            nc.sync.dma_start(vb_sb[:], vbias[:])
            b_sb = consts.tile([128, QKC], F32)
            nc.sync.dma_start(b_sb[:], bqkv.rearrange("(c p) -> p c", p=128))

            # ---------------- phase A: projections ----------------
            with ExitStack() as actx:
                wpool = actx.enter_context(tc.tile_pool(name="w", bufs=1))
                xpool = actx.enter_context(tc.tile_pool(name="x", bufs=1))
                stpool = actx.enter_context(tc.tile_pool(name="stage", bufs=4))
                psA = actx.enter_context(tc.tile_pool(name="psA", bufs=1, space="PSUM"))

                # W and x split per 128-row contraction chunk so matmuls start
                # as soon as the first chunks land. qk weight chunks + first
                # x tile stream first; v weight chunks follow (v runs later).
                wqk_t, wv_t, x_tiles = [], [], {}
                for db in range(DB):
                    w_db = wpool.tile([128, 2 * CG], F32R, tag="wqk", bufs=DB)
                    nc.sync.dma_start(w_db[:], wqkv[db * 128:(db + 1) * 128, 0:2 * CG])
                    wqk_t.append(w_db)
                    x_db = xpool.tile([128, A_ST], F32R, tag="x", bufs=2 * DB)
                    nc.sync.dma_start(x_db[:], xT[db * 128:(db + 1) * 128, 0:A_ST])
                    x_tiles[(0, db)] = x_db
                for db in range(DB):
                    w_db = wpool.tile([128, CG], F32R, tag="wv", bufs=DB)
                    nc.sync.dma_start(
                        w_db[:], wqkv[db * 128:(db + 1) * 128, 2 * CG:3 * CG])
                    wv_t.append(w_db)

                for t in range(A_NT):
                    if t + 1 < A_NT:  # prefetch next s-tile's x chunks
                        for db in range(DB):
                            x_db = xpool.tile([128, A_ST], F32R, tag="x",
                                              bufs=2 * DB)
                            nc.sync.dma_start(
                                x_db[:],
                                xT[db * 128:(db + 1) * 128,
                                   (t + 1) * A_ST:(t + 2) * A_ST])
                            x_tiles[(t + 1, db)] = x_db

                    def qk_stage(cb, ps):
                        st = stpool.tile([128, A_ST], F32R)
                        nc.vector.tensor_scalar_add(st[:], ps[:],
                                                    b_sb[:, cb:cb + 1])
                        nc.sync.dma_start(
                            qk_d[cb][:, t * A_ST:(t + 1) * A_ST], st[:])

                    if t == 0:
                        # chunk-arrival-paced: keep all 8 qk psum chains in
                        # flight so each arriving (w, x) chunk feeds 8 matmuls
                        ps_qk = []
                        for _cb in range(QKC):
                            ps_cb = psA.tile([128, A_ST], F32, tag="ps",
                                             bufs=8, name=f"ps_qk{_cb}")
                            ps_qk.append(ps_cb)
                        for db in range(DB):
                            for cb in range(QKC):
                                nc.tensor.matmul(
                                    ps_qk[cb][:],
                                    lhsT=wqk_t[db][:, cb * 128:cb * 128 + 128],
                                    rhs=x_tiles[(0, db)][:],
                                    start=(db == 0), stop=(db == DB - 1))
                        for cb in range(QKC):
                            qk_stage(cb, ps_qk[cb])
                    else:
                        for cb in range(QKC):
                            ps = psA.tile([128, A_ST], F32, tag="ps", bufs=8)
                            for db in range(DB):
                                nc.tensor.matmul(
                                    ps[:],
                                    lhsT=wqk_t[db][:, cb * 128:cb * 128 + 128],
                                    rhs=x_tiles[(t, db)][:],
                                    start=(db == 0), stop=(db == DB - 1))
                            qk_stage(cb, ps)
                    # v row blocks (plain projection), written straight to SBUF
                    for si in range(A_ST // 128):
                        sblk = t * (A_ST // 128) + si
                        psv = psA.tile([128, CG], F32, tag="ps", bufs=8)
                        for db in range(DB):
                            nc.tensor.matmul(
                                psv[:],
                                lhsT=x_tiles[(t, db)][:, si * 128:si * 128 + 128],
                                rhs=wv_t[db][:],
                                start=(db == 0), stop=(db == DB - 1))
                        nc.vector.tensor_tensor(
                            v_sb[:, sblk * CG:(sblk + 1) * CG],
                            psv[:], vb_sb[:], ALU.add)

            # ------------- phases B+C: attention + output projection -------------
            with ExitStack() as bctx:
                qkh = bctx.enter_context(tc.tile_pool(name="qkh", bufs=1))
                opool = bctx.enter_context(tc.tile_pool(name="oT", bufs=4))
                epool = bctx.enter_context(tc.tile_pool(name="e", bufs=3))
                apool = bctx.enter_context(tc.tile_pool(name="acc", bufs=2))
                rpool = bctx.enter_context(tc.tile_pool(name="rinv", bufs=2))
                wppool = bctx.enter_context(tc.tile_pool(name="wp", bufs=1))
                mpool = bctx.enter_context(tc.tile_pool(name="masks", bufs=1))
                ppool = bctx.enter_context(tc.tile_pool(name="pout", bufs=4))
                psB = bctx.enter_context(tc.tile_pool(name="psB", bufs=1, space="PSUM"))

                mask_sb = mpool.tile([128, 4 * 512], F32R)
                nc.sync.dma_start(
                    mask_sb[:].rearrange("p (r s) -> p r s", r=4),
                    masks.rearrange("(r p) s -> p r s", p=128),
                )
                # all heads' qT/kT resident; loaded in 512-col chunks
                # ordered so B(tau=0) unblocks after ~2MB instead of 13MB
                qT_t, kT_t = [], []
                for h in range(HEADS_PER_CORE):
                    qT_h = qkh.tile([128, S], F32R, tag="qkh", bufs=QKC,
                                    name=f"qT{h}")
                    qT_t.append(qT_h)
                for h in range(HEADS_PER_CORE):
                    kT_h = qkh.tile([128, S], F32R, tag="qkh", bufs=QKC,
                                    name=f"kT{h}")
                    kT_t.append(kT_h)
                for tc_ in range(4):
                    for h in range(HEADS_PER_CORE):
                        sl = slice(tc_ * 512, (tc_ + 1) * 512)
                        nc.sync.dma_start(qT_t[h][:, sl], qk_d[h][:, sl])
                        nc.sync.dma_start(kT_t[h][:, sl],
                                          qk_d[HEADS_PER_CORE + h][:, sl])
                    if tc_ == 0:
                        wp_sb = wppool.tile([128, HEADS_PER_CORE * D], F32R)
                        nc.sync.dma_start(
                            wp_sb[:].rearrange("p (cb d) -> p cb d",
                                               cb=HEADS_PER_CORE),
                            wproj.rearrange("(cb p) d -> p cb d", p=128),
                        )

                oT_tiles = []
                for _h in range(HEADS_PER_CORE):
                    oT_h = opool.tile([128, S], F32R, tag="oT", name=f"oT{_h}")
                    oT_tiles.append(oT_h)

                def emit_c(t):
                    # C for s-tile t (all heads' oT[t] ready)
                    for dblk in range(DB):
                        ps_p = psB.tile([128, 512], F32, tag="pp", bufs=2)
                        for cb in range(HEADS_PER_CORE):
                            nc.tensor.matmul(
                                ps_p[:],
                                lhsT=wp_sb[:, cb * D + dblk * 128:
                                           cb * D + dblk * 128 + 128],
                                rhs=oT_tiles[cb][:, t * 512:(t + 1) * 512],
                                start=(cb == 0),
                                stop=(cb == HEADS_PER_CORE - 1))
                        po = ppool.tile([128, 512], F32)
                        nc.any.tensor_copy(po[:], ps_p[:])
                        nc.sync.dma_start(
                            outT[dblk * 128:(dblk + 1) * 128,
                                 t * 512:(t + 1) * 512], po[:])

                for t in range(4):  # 512-wide query tiles
                    npair = 2 * t + 2  # causal: t-blocks 0..4t+3, in pairs
                    for h in range(HEADS_PER_CORE):
                        if h == 1 and t > 0:
                            emit_c(t - 1)  # delayed so it fills B's stalls
                        ps_o = psB.tile([128, 512], F32, tag="o", bufs=2)
                        acc = apool.tile([128, 1024], F32R, tag="acc")
                        for pj in range(npair):
                            # scores for t-blocks 2pj, 2pj+1 into a paired
                            # 2-bank psum tile; one wide exp over both
                            ps_sc = psB.tile([128, 1024], F32, tag="sc", bufs=2)
                            for half in range(2):
                                j = 2 * pj + half
                                nc.tensor.matmul(
                                    ps_sc[:, half * 512:(half + 1) * 512],
                                    lhsT=kT_t[h][:, j * 128:(j + 1) * 128],
                                    rhs=qT_t[h][:, t * 512:(t + 1) * 512],
                                    start=True, stop=True)
                            e = epool.tile([128, 1024], F32R, tag="e")
                            nc.scalar.activation(e[:], ps_sc[:], ACT.Exp)
                            if pj >= npair - 2:  # diagonal region: causal mask
                                r0 = 2 * (pj - (npair - 2))  # 0 or 2
                                nc.vector.tensor_tensor(
                                    e[:], e[:],
                                    mask_sb[:, r0 * 512:(r0 + 2) * 512],
                                    ALU.mult)
                            if pj == 0:
                                nc.vector.tensor_copy(acc[:], e[:])
                            else:
                                nc.vector.tensor_tensor(acc[:], acc[:], e[:],
                                                        ALU.add)
                            for half in range(2):
                                j = 2 * pj + half
                                nc.tensor.matmul(
                                    ps_o[:],
                                    lhsT=v_sb[:, j * CG + h * 128:
                                              j * CG + h * 128 + 128],
                                    rhs=e[:, half * 512:(half + 1) * 512],
                                    start=(pj == 0 and half == 0),
                                    stop=(pj == npair - 1 and half == 1))
                        ps_l = psB.tile([128, 512], F32, tag="sc", bufs=2)
                        for half in range(2):
                            nc.tensor.matmul(
                                ps_l[:], lhsT=ones_sb[:],
                                rhs=acc[:, half * 512:(half + 1) * 512],
                                start=(half == 0), stop=(half == 1))
                        rinv = rpool.tile([128, 512], F32, tag="rinv")
                        nc.vector.reciprocal(rinv[:], ps_l[:])
                        nc.vector.tensor_tensor(
                            oT_tiles[h][:, t * 512:(t + 1) * 512], ps_o[:],
                            rinv[:], ALU.mult)


                emit_c(3)

    nc.compile()
    return nc


def _get_program():
    global _compiled
    if _compiled is None:
        _compiled = _build_program()
    return _compiled


def _make_in_maps(hidden_states, W_attn, b_attn, W_proj):
    x = np.asarray(hidden_states, dtype=np.float32)
    W_attn = np.asarray(W_attn, dtype=np.float32)
    b_attn = np.asarray(b_attn, dtype=np.float32)
    W_proj = np.asarray(W_proj, dtype=np.float32)

    qs = np.float32(1.0 / np.sqrt(HD))
    # causal 0/1 masks for the 4 diagonal-region block offsets:
    # mask[r][t, s] = 1 if t + 128*r <= s else 0   (t in 0..127, s in 0..511)
    m = np.zeros((4, 128, 512), dtype=np.float32)
    for r in range(4):
        m[r] = (np.arange(128)[:, None] + 128 * r
                <= np.arange(512)[None, :]).astype(np.float32)
    masks_np = np.ascontiguousarray(m.reshape(4 * 128, 512))
    ones_np = np.ones((128, 128), dtype=np.float32)

    in_maps = []
    for core in range(N_CORES):
        b, g = divmod(core, HEADS_PER_CORE)
        c0 = g * CG
        wq = W_attn[:, c0:c0 + CG] * qs
        wk = W_attn[:, D + c0:D + c0 + CG]
        wv = W_attn[:, 2 * D + c0:2 * D + c0 + CG]
        bq = b_attn[c0:c0 + CG] * qs
        bk = b_attn[D + c0:D + c0 + CG]
        bv = b_attn[2 * D + c0:2 * D + c0 + CG]
        in_maps.append({
            "xT": np.ascontiguousarray(x[b].T),
            "wqkv": np.ascontiguousarray(
                np.concatenate([wq, wk, wv], axis=1)),
            "bqkv": np.ascontiguousarray(
                np.concatenate([bq, bk])),
            "vbias": np.ascontiguousarray(
                np.broadcast_to(bv[None, :], (128, CG))),
            "wproj": np.ascontiguousarray(W_proj[c0:c0 + CG, :]),
            "masks": masks_np,
            "ones": ones_np,
        })
    return in_maps


def kernel(hidden_states, W_attn, b_attn, W_proj, b_proj):
    from concourse.bass_utils import run_bass_kernel_spmd

    nc = _get_program()
    in_maps = _make_in_maps(hidden_states, W_attn, b_attn, W_proj)
    res = run_bass_kernel_spmd(nc, in_maps, core_ids=list(range(N_CORES)))

    b_proj = np.asarray(b_proj, dtype=np.float32)
    out = np.empty((B, S, D), dtype=np.float32)
    for b in range(B):
        acc = res.results[b * HEADS_PER_CORE + 0]["outT"].copy()
        for g in range(1, HEADS_PER_CORE):
            acc += res.results[b * HEADS_PER_CORE + g]["outT"]
        out[b] = acc.T + b_proj[None, :]
    return out


# revision 19
# speedup vs baseline: 3.3771x; 3.3771x over previous
"""Trainium2 Bass kernel for a causal multi-head attention layer.

Full (unsharded) contract:
    hidden_states [2, 2048, 2048] f32, W_attn [2048, 6144], b_attn [6144],
    W_proj [2048, 2048], b_proj [2048]  ->  out [2, 2048, 2048] f32

Sharding over 8 NeuronCores: core = b * 4 + g where b in {0,1} is the batch
(data parallel) and g in 0..3 is a group of 4 heads (tensor parallel over
heads: c_attn output columns / c_proj input rows split by head). Each core
computes a full [2048, 2048] c_proj partial (transposed); the host sums the
4 head-group partials per batch and adds b_proj.

On-device dataflow is fully transposed so no transposes are ever required:
  phase A: qT,kT [c, s] = W.T @ xT (transposed projection), v [s, c] (plain
           projection)  -- both consume xT [d, s] and W [d, c] naturally.
  phase B: scoresT [t, s] = kT_blk.T @ qT ; exp ; causal 0/1 mask multiply ;
           outT_un [d, s] += v_blk.T @ expT ; denominator via ones-matmul
           (broadcast column sum) ; outT = outT_un * reciprocal.
  phase C: partialT [Dout, s] = Wp_blk.T @ outT, interleaved with B per
           s-tile so PE/ACT/DVE load stays balanced.
All matmuls are float32r (full PE rate at N>=256, ~1.5e-4 relative error).
"""

from contextlib import ExitStack

import numpy as np

B = 2
S = 2048
D = 2048
NUM_HEAD = 16
HD = 128
N_CORES = 8
HEADS_PER_CORE = 4  # head-group size (512 columns)
CG = HEADS_PER_CORE * HD  # 512

_compiled = None


def _build_program(repeat=1):
    import concourse.bacc as bacc
    import concourse.mybir as mybir
    import concourse.tile as tile

    F32 = mybir.dt.float32
    F32R = mybir.dt.float32r
    ACT = mybir.ActivationFunctionType
    ALU = mybir.AluOpType

    nc = bacc.Bacc("TRN2", target_bir_lowering=False, debug=False)

    xT = nc.dram_tensor("xT", [D, S], F32R, kind="ExternalInput").ap()
    wqkv = nc.dram_tensor("wqkv", [D, 3 * CG], F32R, kind="ExternalInput").ap()
    bqkv = nc.dram_tensor("bqkv", [2 * CG], F32, kind="ExternalInput").ap()
    vbias = nc.dram_tensor("vbias", [128, CG], F32, kind="ExternalInput").ap()
    wproj = nc.dram_tensor("wproj", [CG, D], F32R, kind="ExternalInput").ap()
    masks = nc.dram_tensor("masks", [4 * 128, 512], F32R, kind="ExternalInput").ap()
    ones = nc.dram_tensor("ones", [128, 128], F32R, kind="ExternalInput").ap()
    outT = nc.dram_tensor("outT", [D, S], F32, kind="ExternalOutput").ap()

    DB = D // 128      # 16 contraction blocks
    A_ST = 512         # phase A s-tile width
    A_NT = S // A_ST   # 4
    QKC = 2 * CG // 128  # 8 q/k column blocks of 128

    with tile.TileContext(nc) as tc:
      for _rep in range(repeat):
        with ExitStack() as octx:
            # pools that live across phases
            vpool = octx.enter_context(tc.tile_pool(name="vsb", bufs=1))
            consts = octx.enter_context(tc.tile_pool(name="consts", bufs=1))
            qkh0 = octx.enter_context(tc.tile_pool(name="qkh0", bufs=1))
            mpool = octx.enter_context(tc.tile_pool(name="masks", bufs=1))
            dram = octx.enter_context(tc.tile_pool(name="dram", bufs=1, space="DRAM"))
            # qk_sb[(cb, tc)] = SBUF chunk [128, 512] of qT/kT row block cb,
            # columns tc*512:.. ; tau=0 chunks are loaded during phase A
            qk_sb = {}

            v_sb = vpool.tile([128, DB * CG], F32R)  # [s%128, sblk-major x c] 4MB
            # per-column-block DRAM staging for qT/kT rows (fine-grained deps
            # so phase B loads can start as soon as each row block completes)
            qk_d = []
            for cb in range(QKC):
                qk_cb = dram.tile([128, S], F32R, tag=f"qk{cb}", name=f"qk_d{cb}")
                qk_d.append(qk_cb)

            ones_sb = consts.tile([128, 128], F32R)
            nc.sync.dma_start(ones_sb[:], ones[:])
            vb_sb = consts.tile([128, CG], F32)
            nc.sync.dma_start(vb_sb[:], vbias[:])
            b_sb = consts.tile([128, QKC], F32)
            nc.sync.dma_start(b_sb[:], bqkv.rearrange("(c p) -> p c", p=128))

            # ---------------- phase A: projections ----------------
            with ExitStack() as actx:
                wpool = actx.enter_context(tc.tile_pool(name="w", bufs=1))
                xpool = actx.enter_context(tc.tile_pool(name="x", bufs=1))
                stpool = actx.enter_context(tc.tile_pool(name="stage", bufs=3))
                psA = actx.enter_context(tc.tile_pool(name="psA", bufs=1, space="PSUM"))

                # W and x split per 128-row contraction chunk so matmuls start
                # as soon as the first chunks land. qk weight chunks + first
                # x tile stream first; v weight chunks follow (v runs later).
                wqk_t, wv_t, x_tiles = [], [], {}
                for db in range(DB):
                    w_db = wpool.tile([128, 2 * CG], F32R, tag="wqk", bufs=DB)
                    nc.sync.dma_start(w_db[:], wqkv[db * 128:(db + 1) * 128, 0:2 * CG])
                    wqk_t.append(w_db)
                    x_db = xpool.tile([128, A_ST], F32R, tag="x", bufs=20)
                    nc.sync.dma_start(x_db[:], xT[db * 128:(db + 1) * 128, 0:A_ST])
                    x_tiles[(0, db)] = x_db
                for db in range(DB):
                    w_db = wpool.tile([128, CG], F32R, tag="wv", bufs=DB)
                    nc.sync.dma_start(
                        w_db[:], wqkv[db * 128:(db + 1) * 128, 2 * CG:3 * CG])
                    wv_t.append(w_db)

                for t in range(A_NT):
                    if t + 1 < A_NT:  # prefetch next s-tile's x chunks
                        for db in range(DB):
                            x_db = xpool.tile([128, A_ST], F32R, tag="x",
                                              bufs=20)
                            nc.sync.dma_start(
                                x_db[:],
                                xT[db * 128:(db + 1) * 128,
                                   (t + 1) * A_ST:(t + 2) * A_ST])
                            x_tiles[(t + 1, db)] = x_db

                    def qk_stage(cb, ps):
                        st = stpool.tile([128, A_ST], F32R)
                        nc.vector.tensor_scalar_add(st[:], ps[:],
                                                    b_sb[:, cb:cb + 1])
                        nc.sync.dma_start(
                            qk_d[cb][:, t * A_ST:(t + 1) * A_ST], st[:])
                        if t == 0:  # prefetch B's tau=0 chunk during A
                            ch = qkh0.tile([128, 512], F32R, tag="qkh0",
                                           bufs=QKC, name=f"qk0_{cb}")
                            nc.sync.dma_start(ch[:], qk_d[cb][:, 0:512])
                            qk_sb[(cb, 0)] = ch

                    if t == 0:
                        # chunk-arrival-paced: keep all 8 qk psum chains in
                        # flight so each arriving (w, x) chunk feeds 8 matmuls
                        ps_qk = []
                        for _cb in range(QKC):
                            ps_cb = psA.tile([128, A_ST], F32, tag="ps",
                                             bufs=8, name=f"ps_qk{_cb}")
                            ps_qk.append(ps_cb)
                        for db in range(DB):
                            for cb in range(QKC):
                                nc.tensor.matmul(
                                    ps_qk[cb][:],
                                    lhsT=wqk_t[db][:, cb * 128:cb * 128 + 128],
                                    rhs=x_tiles[(0, db)][:],
                                    start=(db == 0), stop=(db == DB - 1))
                        for cb in range(QKC):
                            qk_stage(cb, ps_qk[cb])
                    else:
                        for cb in range(QKC):
                            ps = psA.tile([128, A_ST], F32, tag="ps", bufs=8)
                            for db in range(DB):
                                nc.tensor.matmul(
                                    ps[:],
                                    lhsT=wqk_t[db][:, cb * 128:cb * 128 + 128],
                                    rhs=x_tiles[(t, db)][:],
                                    start=(db == 0), stop=(db == DB - 1))
                            qk_stage(cb, ps)
                    if t == 0:
                        mask_sb = mpool.tile([128, 4 * 512], F32R)
                        nc.sync.dma_start(
                            mask_sb[:].rearrange("p (r s) -> p r s", r=4),
                            masks.rearrange("(r p) s -> p r s", p=128),
                        )
                    # v row blocks (plain projection), written straight to SBUF
                    for si in range(A_ST // 128):
                        sblk = t * (A_ST // 128) + si
                        psv = psA.tile([128, CG], F32, tag="ps", bufs=8)
                        for db in range(DB):
                            nc.tensor.matmul(
                                psv[:],
                                lhsT=x_tiles[(t, db)][:, si * 128:si * 128 + 128],
                                rhs=wv_t[db][:],
                                start=(db == 0), stop=(db == DB - 1))
                        nc.vector.tensor_tensor(
                            v_sb[:, sblk * CG:(sblk + 1) * CG],
                            psv[:], vb_sb[:], ALU.add)

            # ------------- phases B+C: attention + output projection -------------
            with ExitStack() as bctx:
                qkh = bctx.enter_context(tc.tile_pool(name="qkh", bufs=1))
                opool = bctx.enter_context(tc.tile_pool(name="oT", bufs=4))
                epool = bctx.enter_context(tc.tile_pool(name="e", bufs=3))
                apool = bctx.enter_context(tc.tile_pool(name="acc", bufs=2))
                rpool = bctx.enter_context(tc.tile_pool(name="rinv", bufs=2))
                wppool = bctx.enter_context(tc.tile_pool(name="wp", bufs=1))
                ppool = bctx.enter_context(tc.tile_pool(name="pout", bufs=4))
                psB = bctx.enter_context(tc.tile_pool(name="psB", bufs=1, space="PSUM"))

                # remaining qT/kT chunks (tau=1..3); tau=0 was prefetched
                # during phase A. Order: tau=1, wp, tau=2, tau=3.
                for tc_ in range(1, 4):
                    for cb in range(QKC):
                        sl = slice(tc_ * 512, (tc_ + 1) * 512)
                        ch = qkh.tile([128, 512], F32R, tag="qkh123",
                                      bufs=3 * QKC, name=f"qk{tc_}_{cb}")
                        nc.sync.dma_start(ch[:], qk_d[cb][:, sl])
                        qk_sb[(cb, tc_)] = ch
                    if tc_ == 1:
                        wp_sb = wppool.tile([128, HEADS_PER_CORE * D], F32R)
                        nc.sync.dma_start(
                            wp_sb[:].rearrange("p (cb d) -> p cb d",
                                               cb=HEADS_PER_CORE),
                            wproj.rearrange("(cb p) d -> p cb d", p=128),
                        )

                oT_tiles = []
                for _h in range(HEADS_PER_CORE):
                    oT_h = opool.tile([128, S], F32R, tag="oT", name=f"oT{_h}")
                    oT_tiles.append(oT_h)

                def emit_c(t, dblks=range(DB)):
                    # C for s-tile t (all heads' oT[t] ready)
                    for dblk in dblks:
                        ps_p = psB.tile([128, 512], F32, tag="pp", bufs=2)
                        for cb in range(HEADS_PER_CORE):
                            nc.tensor.matmul(
                                ps_p[:],
                                lhsT=wp_sb[:, cb * D + dblk * 128:
                                           cb * D + dblk * 128 + 128],
                                rhs=oT_tiles[cb][:, t * 512:(t + 1) * 512],
                                start=(cb == 0),
                                stop=(cb == HEADS_PER_CORE - 1))
                        po = ppool.tile([128, 512], F32)
                        nc.any.tensor_copy(po[:], ps_p[:])
                        nc.sync.dma_start(
                            outT[dblk * 128:(dblk + 1) * 128,
                                 t * 512:(t + 1) * 512], po[:])

                for t in range(4):  # 512-wide query tiles
                    npair = 2 * t + 2  # causal: t-blocks 0..4t+3, in pairs
                    c_spread = [(0, 6), (6, 11), (11, 16)]
                    for h in range(HEADS_PER_CORE):
                        if h > 0 and t > 0:  # spread so it fills B's stalls
                            lo, hi = c_spread[h - 1]
                            emit_c(t - 1, range(lo, hi))
                        ps_o = psB.tile([128, 512], F32, tag="o", bufs=2)
                        acc = apool.tile([128, 1024], F32R, tag="acc")
                        # masked (diagonal) pairs first so their extra DVE
                        # mask op is off the tail of the acc chain
                        pj_order = [npair - 2, npair - 1] + list(range(npair - 2))
                        for pi, pj in enumerate(pj_order):
                            # scores for t-blocks 2pj, 2pj+1 into a paired
                            # 2-bank psum tile; one wide exp over both
                            ps_sc = psB.tile([128, 1024], F32, tag="sc", bufs=2)
                            for half in range(2):
                                j = 2 * pj + half
                                ktc = qk_sb[(HEADS_PER_CORE + h, j // 4)]
                                nc.tensor.matmul(
                                    ps_sc[:, half * 512:(half + 1) * 512],
                                    lhsT=ktc[:, (j % 4) * 128:(j % 4) * 128 + 128],
                                    rhs=qk_sb[(h, t)][:],
                                    start=True, stop=True)
                            e = epool.tile([128, 1024], F32R, tag="e")
                            nc.scalar.activation(e[:], ps_sc[:], ACT.Exp)
                            if pj >= npair - 2:  # diagonal region: causal mask
                                r0 = 2 * (pj - (npair - 2))  # 0 or 2
                                nc.vector.tensor_tensor(
                                    e[:], e[:],
                                    mask_sb[:, r0 * 512:(r0 + 2) * 512],
                                    ALU.mult)
                            if pi == 0:
                                nc.vector.tensor_copy(acc[:], e[:])
                            else:
                                nc.vector.tensor_tensor(acc[:], acc[:], e[:],
                                                        ALU.add)
                            for half in range(2):
                                j = 2 * pj + half
                                nc.tensor.matmul(
                                    ps_o[:],
                                    lhsT=v_sb[:, j * CG + h * 128:
                                              j * CG + h * 128 + 128],
                                    rhs=e[:, half * 512:(half + 1) * 512],
                                    start=(pi == 0 and half == 0),
                                    stop=(pi == npair - 1 and half == 1))
                        ps_l = psB.tile([128, 512], F32, tag="sc", bufs=2)
                        for half in range(2):
                            nc.tensor.matmul(
                                ps_l[:], lhsT=ones_sb[:],
                                rhs=acc[:, half * 512:(half + 1) * 512],
                                start=(half == 0), stop=(half == 1))
                        rinv = rpool.tile([128, 512], F32, tag="rinv")
                        nc.vector.reciprocal(rinv[:], ps_l[:])
                        nc.vector.tensor_tensor(
                            oT_tiles[h][:, t * 512:(t + 1) * 512], ps_o[:],
                            rinv[:], ALU.mult)


                emit_c(3)

    nc.compile()
    return nc


def _get_program():
    global _compiled
    if _compiled is None:
        _compiled = _build_program()
    return _compiled


def _make_in_maps(hidden_states, W_attn, b_attn, W_proj):
    x = np.asarray(hidden_states, dtype=np.float32)
    W_attn = np.asarray(W_attn, dtype=np.float32)
    b_attn = np.asarray(b_attn, dtype=np.float32)
    W_proj = np.asarray(W_proj, dtype=np.float32)

    qs = np.float32(1.0 / np.sqrt(HD))
    # causal 0/1 masks for the 4 diagonal-region block offsets:
    # mask[r][t, s] = 1 if t + 128*r <= s else 0   (t in 0..127, s in 0..511)
    m = np.zeros((4, 128, 512), dtype=np.float32)
    for r in range(4):
        m[r] = (np.arange(128)[:, None] + 128 * r
                <= np.arange(512)[None, :]).astype(np.float32)
    masks_np = np.ascontiguousarray(m.reshape(4 * 128, 512))
    ones_np = np.ones((128, 128), dtype=np.float32)

    in_maps = []
    for core in range(N_CORES):
        b, g = divmod(core, HEADS_PER_CORE)
        c0 = g * CG
        wq = W_attn[:, c0:c0 + CG] * qs
        wk = W_attn[:, D + c0:D + c0 + CG]
        wv = W_attn[:, 2 * D + c0:2 * D + c0 + CG]
        bq = b_attn[c0:c0 + CG] * qs
        bk = b_attn[D + c0:D + c0 + CG]
        bv = b_attn[2 * D + c0:2 * D + c0 + CG]
        in_maps.append({
            "xT": np.ascontiguousarray(x[b].T),
            "wqkv": np.ascontiguousarray(
                np.concatenate([wq, wk, wv], axis=1)),
            "bqkv": np.ascontiguousarray(
                np.concatenate([bq, bk])),
            "vbias": np.ascontiguousarray(
                np.broadcast_to(bv[None, :], (128, CG))),
            "wproj": np.ascontiguousarray(W_proj[c0:c0 + CG, :]),
            "masks": masks_np,
            "ones": ones_np,
        })
    return in_maps


def kernel(hidden_states, W_attn, b_attn, W_proj, b_proj):
    from concourse.bass_utils import run_bass_kernel_spmd

    nc = _get_program()
    in_maps = _make_in_maps(hidden_states, W_attn, b_attn, W_proj)
    res = run_bass_kernel_spmd(nc, in_maps, core_ids=list(range(N_CORES)))

    b_proj = np.asarray(b_proj, dtype=np.float32)
    out = np.empty((B, S, D), dtype=np.float32)
    for b in range(B):
        acc = res.results[b * HEADS_PER_CORE + 0]["outT"].copy()
        for g in range(1, HEADS_PER_CORE):
            acc += res.results[b * HEADS_PER_CORE + g]["outT"]
        out[b] = acc.T + b_proj[None, :]
    return out
